# revision 15
# baseline (speedup 1.0000x reference)
"""Bass/Trainium2 kernel for nn_DFTLayer: out[b,f,k] = DFT_1024(x[b,f,:]).

reference: real = einsum('bfs,ks->bfk', x, wcos); imag = ... wsin
           out  = complex(real, -imag),  x: [16, 1024, 1024] f32.

Strategy (8 NeuronCores, data-parallel over batch, 2 batches/core):
  - Hermitian symmetry (x real): out[k] = conj(out[N-k]); device covers
    k = 0..255 (and k = 257..512 via butterflies); col 256 and the
    k = 513..1023 mirror are host-side.
  - Cosine/sine parity fold (host): u[s] = x[s] + x[N-s], v[s] = x[s] - x[N-s]
    over contraction slots s = 1..512 (u[512] = x[512], v[512] coeff is 0):
        real[k] = x[0] + sum_{s=1..512} u[s] cos(2*pi*k*s/N)
        imag[k] =        sum_{s=1..511} v[s] sin(2*pi*k*s/N)
  - Radix-2 split by parity of s (host): ue[t] = u[2t+2], uo[t] = u[2t+1]
    (t = 0..255), likewise ve/vo:
        E[k] = ue @ cos(2pi k(2t+2)/N),  O[k] = uo @ cos(2pi k(2t+1)/N)
        real[k] = x0 + E[k] + O[k];  real[512-k] = x0 + E[k] - O[k]
        (imag via Es/Os with sin; imag[512-k] = -Es[k] + Os[k])
  - Second split on the EVEN branches only: uea[r] = ue[2r], ueb[r] = ue[2r+1]:
        Ea[k] = uea @ cos(2pi k(4r+2)/N), Eb[k] = ueb @ cos(2pi k(r+1)/256)
        E[k] = Ea[k] + Eb[k];   E[256-k] = -Ea[k] + Eb[k]   (k = 0..127)
        E[128], Es[128]: host dot products.
    Device matmul work: O/Os at 256-contraction + Ea/Eb/Esa/Esb at 128 =
    24576 PE cycles (~10.3 us at 2.4 GHz).
  - Everything crossing HBM is bf16: ~8.4 MB per core; rel err ~3e-3.
  - DMA/schedule plan (v4, from trace analysis of v1-v3):
      * All queues share the 16 SDMA engines; aggregate tops at ~0.41
        MB/us. Total bytes (8.4 MB) / 0.41 is the hard streaming floor;
        the only other levers are the ~8.3us preamble-to-first-byte,
        ring bubbles, and the epilogue.
      * Everything rides ONE DRAM tensor ("uv", partition-major: each
        partition's bytes contiguous) packed in stream order
        [wE | uea | ueb | wO | uo | vo | vea | veb]; 5 merged transfers
        on the sync ring -> 128 big descriptors each, minimal gaps.
      * Phase order Ea, Eb, O, O, Os, Os, Esa, Esb: first matmul needs
        only 0.63 MB; the PE's pre-HAM-grant half-clock era (until
        ~19us) is spent on real work, not idle waiting for 1.9 MB.
      * All outputs are casted into an 8-deep SBUF buffer pool as soon
        as PSUM fills, and drain FIFO behind the inputs on the same
        sync ring -> the ring never bubbles and the drain (pure DMA) is
        immune to the HAM half-duty tail. Last phase's second half
        drains on the scalar ring in parallel.
  - PE p-state warm-up via memset-fed dummy matmuls.
  - PSUM -> SBUF bf16 casts split ACT/DVE per row-half (only they can
    read PSUM). Butterflies/mirrors/corrections happen on the host.
"""

import sys

for _p in ("/opt/trn_rl_repo", "/root/.axon_site/_ro/trn_rl_repo"):
    if _p not in sys.path:
        sys.path.append(_p)

import numpy as np
import ml_dtypes
from contextlib import ExitStack

BF16 = np.dtype(ml_dtypes.bfloat16)

N_CORES = 8
B, F_FULL, S = 16, 1024, 1024          # x: [B, F_FULL, S]
F = (B // N_CORES) * F_FULL            # 2048 rows per core
M = 256                                # radix-2 contraction length
KD = 256                               # freq cols per level-1 kernel
WARMUP_MM = 8                          # dummy matmuls to ramp the PE p-state

# packed input column map (bf16 cols of the [128, NCOL] "uv" tensor):
C_WE = 0          # [Ea|Eb|Esa|Esb] 128-contraction kernels   [128, 512]
C_WO = 512        # O kernels [t*512 + kern2*256 + kc*128 + q] [128, 1024]
C_B0 = 1536      # uo half 0 (rows 0..1023)
C_B1 = 3584      # uo half 1
C_B2 = 5632      # uea
C_B3 = 7680      # ueb
C_B4 = 9728      # vo half 0
C_B5 = 11776     # vo half 1
C_B6 = 13824     # vea
C_B7 = 15872     # veb
NCOL = 17920
# merged stream transfers (consumption order): the O phases (longest PE
# chain) get their operands first so the PE's pre-HAM-grant half-clock era
# is spent on the heavy phases
XFERS = [(C_WE, C_B2), (C_B2, C_B4), (C_B4, C_B6), (C_B6, NCOL)]
# eo output slots (bf16 [128, 2048] each): 0/1 = O (k 0..127 / 128..255),
#   2 = Ea, 3 = Eb, 4/5 = Os, 6 = Esa, 7 = Esb; cols = half*1024 + g*512 + j

_CACHE = {}


def _build():
    """Build + compile the per-core Bass program (cached)."""
    if "nc" in _CACHE:
        return _CACHE["nc"]

    from concourse import bacc, tile, mybir

    f32 = mybir.dt.float32
    bf16 = mybir.dt.bfloat16

    nc = bacc.Bacc("TRN2", target_bir_lowering=False, debug=False)

    uv_d = nc.dram_tensor("uv", [128, NCOL], bf16, kind="ExternalInput")
    eo_d = nc.dram_tensor("eo", [128, 8 * 2048], bf16, kind="ExternalOutput")

    with tile.TileContext(nc) as tc, ExitStack() as ctx:
        wpool = ctx.enter_context(tc.tile_pool(name="w", bufs=1))
        opool = ctx.enter_context(tc.tile_pool(name="o", bufs=8))
        ppool = ctx.enter_context(tc.tile_pool(name="p", bufs=4, space="PSUM"))

        f32r = mybir.dt.float32r

        # warm-up operand needs no DMA: memset lands right after the prologue
        wu_t = wpool.tile([128, 512], f32, tag="wu")
        nc.gpsimd.memset(wu_t[:], 1.0)

        uv_t = wpool.tile([128, NCOL], bf16, tag="uv")
        for c0, c1 in XFERS:
            nc.sync.dma_start(uv_t[:, c0:c1], uv_d[:, c0:c1])

        # p-state warm-up: dummy matmuls keep the PE continuously busy from
        # the prologue until real operands arrive. The warm-up PSUM tile is
        # reused as the first phase's first accumulator (same engine, program
        # order) so the 4 PSUM double-buffers map 1:1 onto phase tiles and
        # phase p+2 only ever waits on phase p's casts.
        ps_w = ppool.tile([128, 2, 512], f32, tag="ps")
        for i in range(WARMUP_MM):
            nc.tensor.matmul(ps_w[:, i % 2, 0:128], wu_t[:, 0:128].bitcast(f32r),
                             wu_t[:, 0:128].bitcast(f32r), start=True, stop=True)

        def finish_phase(slot, pss, split):
            """Casts + output DMA for one phase's two PSUM tiles."""
            out_t = opool.tile([128, 2, 2, 512], bf16, tag="out")
            c0 = slot * 2048
            for half in range(2):
                ps = pss[half]
                if half == 0:
                    nc.scalar.copy(out_t[:, half], ps[:])
                else:
                    nc.vector.tensor_copy(out_t[:, half], ps[:])
            if split:
                # tail phase: halves drain on both rings in parallel
                nc.sync.dma_start(eo_d[:, c0:c0 + 1024], out_t[:, 0])
                nc.scalar.dma_start(eo_d[:, c0 + 1024:c0 + 2048], out_t[:, 1])
            else:
                # one [128, 2048] transfer -> 4KB descriptors (2KB ones
                # drain ~25% slower)
                nc.sync.dma_start(eo_d[:, c0:c0 + 2048], out_t[:])

        def e_phase(kern2, sub, split=False):
            """Ea/Eb (or Esa/Esb): one 128-contraction sub-kernel."""
            slot = kern2 * 4 + 2 + sub
            cb = (C_B2, C_B3, C_B6, C_B7)[kern2 * 2 + sub]
            lhsT = uv_t[:, C_WE + (kern2 * 2 + sub) * 128:
                        C_WE + (kern2 * 2 + sub + 1) * 128]
            pss = [ppool.tile([128, 2, 512], f32, tag="ps",
                              name=f"ps_e{slot}_{h}") for h in range(2)]
            for half in range(2):
                for g in range(2):
                    nc.tensor.matmul(
                        pss[half][:, g, :],
                        lhsT,
                        uv_t[:, cb + half * 1024 + g * 512:
                             cb + half * 1024 + (g + 1) * 512],
                        start=True,
                        stop=True,
                    )
            finish_phase(slot, pss, split)

        def o_phase(kern2, kc, first=False):
            """O (or Os): 256-contraction, accumulated over two t-halves."""
            slot = kern2 * 4 + kc
            cb = C_B0 if kern2 == 0 else C_B4
            if first:
                pss = [ps_w,
                       ppool.tile([128, 2, 512], f32, tag="ps", name="ps_o0_1")]
            else:
                pss = [ppool.tile([128, 2, 512], f32, tag="ps",
                                  name=f"ps_o{slot}_{h}") for h in range(2)]
            for t in range(2):
                lhsT = uv_t[:, C_WO + t * 512 + kern2 * 256 + kc * 128:
                            C_WO + t * 512 + kern2 * 256 + (kc + 1) * 128]
                for half in range(2):
                    for g in range(2):
                        nc.tensor.matmul(
                            pss[half][:, g, :],
                            lhsT,
                            uv_t[:, cb + half * 2048 + t * 1024 + g * 512:
                                 cb + half * 2048 + t * 1024 + (g + 1) * 512],
                            start=(t == 0),
                            stop=(t == 1),
                        )
            finish_phase(slot, pss, split=False)

        o_phase(0, 0, first=True)  # O k0  (slot 0) -- heavy phases first
        o_phase(0, 1)            # O k1    (slot 1)
        e_phase(0, 0)            # Ea      (slot 2)
        e_phase(0, 1)            # Eb      (slot 3)
        o_phase(1, 0)            # Os k0   (slot 4)
        o_phase(1, 1)            # Os k1   (slot 5)
        e_phase(1, 0)            # Esa     (slot 6)
        e_phase(1, 1, split=True)  # Esb   (slot 7) -- tail on both rings

    nc.compile()
    _CACHE["nc"] = nc
    return nc


def kernel(x, wsin, wcos):
    from concourse.bass_utils import run_bass_kernel_spmd

    x = np.asarray(x, dtype=np.float32)
    wsin = np.asarray(wsin, dtype=np.float32)
    wcos = np.asarray(wcos, dtype=np.float32)

    nc = _build()

    # DFT kernels sliced from the provided (symmetric) matrices
    wO = wcos[1:512:2, 0:KD]            # cos, s = 2t+1      [256, 256]
    wOs = wsin[1:512:2, 0:KD]           # sin, s = 2t+1
    cEa = wcos[2:512:4, 0:128]          # cos, s = 4r+2      [128, 128]
    cEb = wcos[4:513:4, 0:128]          # cos, s = 4r+4
    sEa = wsin[2:512:4, 0:128]
    sEb = wsin[4:513:4, 0:128]
    wE_np = np.concatenate([cEa, cEb, sEa, sEb], axis=1).astype(BF16)
    oo = np.concatenate([wO, wOs], axis=1).astype(BF16)         # [256, 512]
    wO_np = np.ascontiguousarray(
        oo.reshape(2, 128, 512).transpose(1, 0, 2).reshape(128, 1024))

    # host fold + parity split (f32), then bf16
    xa = x[:, :, 1:512]
    xb = x[:, :, 1023:512:-1]
    u = xa + xb                         # u[s], s = 1..511
    v = xa - xb
    uvp = np.empty((B, F_FULL, 4, M), dtype=np.float32)
    uvp[:, :, 0, :255] = u[:, :, 1::2]  # ue: s = 2,4,..,510
    uvp[:, :, 0, 255] = x[:, :, 512]    # ue[255] <- u[512] = x[512]
    uvp[:, :, 1, :] = u[:, :, 0::2]     # uo: s = 1,3,..,511
    uvp[:, :, 2, :255] = v[:, :, 1::2]  # ve
    uvp[:, :, 2, 255] = 0.0
    uvp[:, :, 3, :] = v[:, :, 0::2]     # vo
    uvp_bf = uvp.astype(BF16)

    bpc = B // N_CORES
    in_maps = []
    for c in range(N_CORES):
        blk = uvp_bf[c * bpc:(c + 1) * bpc].reshape(F, 4, M)
        uv_c = np.empty((8, 128, F), dtype=BF16)
        for kern2, (iodd, ieven) in enumerate(((1, 0), (3, 2))):
            bb = kern2 * 4
            # odd-s data: [row, t] -> blocks [h][p, (tc, j)]
            ot = np.ascontiguousarray(blk[:, iodd, :].T)      # [256, F]
            uv_c[bb:bb + 2] = ot.reshape(2, 128, 2, 1024).transpose(
                2, 1, 0, 3).reshape(2, 128, 2048)
            # even-s data split by parity of t: flat [r, rows]
            ev = blk[:, ieven, :]                             # [F, 256]
            uv_c[bb + 2] = ev[:, 0::2].T                      # a: t = 2r
            uv_c[bb + 3] = ev[:, 1::2].T                      # b: t = 2r+1
        packed = np.concatenate(
            [wE_np, wO_np, uv_c[0], uv_c[1], uv_c[2], uv_c[3],
             uv_c[4], uv_c[5], uv_c[6], uv_c[7]], axis=1)
        in_maps.append({"uv": np.ascontiguousarray(packed)})

    res = run_bass_kernel_spmd(
        nc, in_maps, core_ids=list(range(N_CORES)), **_CACHE.get("run_kwargs", {})
    )
    kernel.last_results = res

    # host assembly: level-2 then level-1 butterflies, x[0] correction,
    # col 256, Hermitian mirror
    alt = np.where(np.arange(M) % 2 == 0, np.float32(1.0), np.float32(-1.0))
    altB = np.where(np.arange(128) % 2 == 0, np.float32(1.0),
                    np.float32(-1.0))
    out = np.empty((B, F_FULL, S), dtype=np.complex64)
    fv = out.view(np.float32).reshape(B, F_FULL, 2 * S)
    for c in range(N_CORES):
        b0 = c * bpc
        eo = np.asarray(res.results[c]["eo"]).reshape(
            128, 8, F).transpose(1, 0, 2)                     # [slot, k, row]
        O = np.concatenate([eo[0], eo[1]]).T.astype(np.float32)   # [F, 256]
        Ea = eo[2].T.astype(np.float32)                           # [F, 128]
        Eb = eo[3].T.astype(np.float32)
        Os = np.concatenate([eo[4], eo[5]]).T.astype(np.float32)
        Esa = eo[6].T.astype(np.float32)
        Esb = eo[7].T.astype(np.float32)
        blk32 = uvp[b0:b0 + bpc].reshape(F, 4, M)
        # level-2 butterflies: rebuild E, Es (k = 0..255)
        E = np.empty((F, 256), dtype=np.float32)
        E[:, 0:128] = Ea + Eb
        E[:, 129:256] = (Eb - Ea)[:, 127:0:-1]
        E[:, 128] = -(blk32[:, 0, 1::2] @ altB)
        Es = np.empty((F, 256), dtype=np.float32)
        Es[:, 0:128] = Esa + Esb
        Es[:, 129:256] = (Esa - Esb)[:, 127:0:-1]
        Es[:, 128] = blk32[:, 2, 0::2] @ altB
        # level-1 butterflies
        x0 = x[b0:b0 + bpc, :, 0].reshape(F, 1)
        reA = E + O
        reA += x0
        reB = E - O
        reB += x0
        imA = Es + Os
        np.negative(imA, out=imA)           # out.imag = -imag_raw
        imB = Es - Os
        fvb = fv[b0:b0 + bpc].reshape(F, 2 * S)
        fvb[:, 0:2 * KD:2] = reA            # real, k = 0..255
        fvb[:, 1:2 * KD:2] = imA
        fvb[:, 514:1026:2] = reB[:, ::-1]   # real, k = 257..512
        fvb[:, 515:1027:2] = imB[:, ::-1]
        # col 256: even-s cos run is (-1)^(t+1), odd-s sin run is (-1)^t
        fvb[:, 512] = x0[:, 0] - blk32[:, 0, :] @ alt
        fvb[:, 513] = -(blk32[:, 3, :] @ alt)
        # Hermitian mirror: out[k] = conj(out[1024-k]) for k = 513..1023
        fvb[:, 1026::2] = fvb[:, 1022:0:-2]
        fvb[:, 1027::2] = -fvb[:, 1023:1:-2]
    return out


# revision 16
# speedup vs baseline: 1.0600x; 1.0600x over previous
"""Bass/Trainium2 kernel for nn_DFTLayer: out[b,f,k] = DFT_1024(x[b,f,:]).

reference: real = einsum('bfs,ks->bfk', x, wcos); imag = ... wsin
           out  = complex(real, -imag),  x: [16, 1024, 1024] f32.

Strategy (8 NeuronCores, data-parallel over batch, 2 batches/core):
  - Hermitian symmetry (x real): out[k] = conj(out[N-k]); device covers
    k = 0..255 (and k = 257..512 via butterflies); col 256 and the
    k = 513..1023 mirror are host-side.
  - Cosine/sine parity fold (host): u[s] = x[s] + x[N-s], v[s] = x[s] - x[N-s]
    over contraction slots s = 1..512 (u[512] = x[512], v[512] coeff is 0):
        real[k] = x[0] + sum_{s=1..512} u[s] cos(2*pi*k*s/N)
        imag[k] =        sum_{s=1..511} v[s] sin(2*pi*k*s/N)
  - Radix-2 split by parity of s (host): ue[t] = u[2t+2], uo[t] = u[2t+1]
    (t = 0..255), likewise ve/vo:
        E[k] = ue @ cos(2pi k(2t+2)/N),  O[k] = uo @ cos(2pi k(2t+1)/N)
        real[k] = x0 + E[k] + O[k];  real[512-k] = x0 + E[k] - O[k]
        (imag via Es/Os with sin; imag[512-k] = -Es[k] + Os[k])
  - Second split on the EVEN branches only: uea[r] = ue[2r], ueb[r] = ue[2r+1]:
        Ea[k] = uea @ cos(2pi k(4r+2)/N), Eb[k] = ueb @ cos(2pi k(r+1)/256)
        E[k] = Ea[k] + Eb[k];   E[256-k] = -Ea[k] + Eb[k]   (k = 0..127)
        E[128], Es[128]: host dot products.
    Device matmul work: O/Os at 256-contraction + Ea/Eb/Esa/Esb at 128 =
    24576 PE cycles (~10.3 us at 2.4 GHz).
  - Everything crossing HBM is bf16: ~8.4 MB per core; rel err ~3e-3.
  - DMA/schedule plan (v4, from trace analysis of v1-v3):
      * All queues share the 16 SDMA engines; aggregate tops at ~0.41
        MB/us. Total bytes (8.4 MB) / 0.41 is the hard streaming floor;
        the only other levers are the ~8.3us preamble-to-first-byte,
        ring bubbles, and the epilogue.
      * Everything rides ONE DRAM tensor ("uv", partition-major: each
        partition's bytes contiguous) packed in stream order
        [wE | uea | ueb | wO | uo | vo | vea | veb]; 5 merged transfers
        on the sync ring -> 128 big descriptors each, minimal gaps.
      * Phase order Ea, Eb, O, O, Os, Os, Esa, Esb: first matmul needs
        only 0.63 MB; the PE's pre-HAM-grant half-clock era (until
        ~19us) is spent on real work, not idle waiting for 1.9 MB.
      * All outputs are casted into an 8-deep SBUF buffer pool as soon
        as PSUM fills, and drain FIFO behind the inputs on the same
        sync ring -> the ring never bubbles and the drain (pure DMA) is
        immune to the HAM half-duty tail. Last phase's second half
        drains on the scalar ring in parallel.
  - PE p-state warm-up via memset-fed dummy matmuls.
  - PSUM -> SBUF bf16 casts split ACT/DVE per row-half (only they can
    read PSUM). Butterflies/mirrors/corrections happen on the host.
"""

import sys

for _p in ("/opt/trn_rl_repo", "/root/.axon_site/_ro/trn_rl_repo"):
    if _p not in sys.path:
        sys.path.append(_p)

import numpy as np
import ml_dtypes
from contextlib import ExitStack

BF16 = np.dtype(ml_dtypes.bfloat16)

N_CORES = 8
B, F_FULL, S = 16, 1024, 1024          # x: [B, F_FULL, S]
F = (B // N_CORES) * F_FULL            # 2048 rows per core
M = 256                                # radix-2 contraction length
KD = 256                               # freq cols per level-1 kernel
WARMUP_MM = 8                          # dummy matmuls to ramp the PE p-state

# packed input column map (bf16 cols of the [128, NCOL] "uv" tensor):
C_WE = 0          # [Ea|Eb|Esa|Esb] 128-contraction kernels   [128, 512]
C_WO = 512        # O kernels [t*512 + kern2*256 + kc*128 + q] [128, 1024]
C_B0 = 1536      # uo half 0 (rows 0..1023)
C_B1 = 3584      # uo half 1
C_B2 = 5632      # uea
C_B3 = 7680      # ueb
C_B4 = 9728      # vo half 0
C_B5 = 11776     # vo half 1
C_B6 = 13824     # vea
C_B7 = 15872     # veb
NCOL = 17920
# merged stream transfers (consumption order): the O phases (longest PE
# chain) get their operands first so the PE's pre-HAM-grant half-clock era
# is spent on the heavy phases
XFERS = [(C_WE, C_B2), (C_B2, C_B4), (C_B4, C_B6), (C_B6, NCOL)]
# eo output slots (bf16 [128, 2048] each): 0/1 = O (k 0..127 / 128..255),
#   2 = Ea, 3 = Eb, 4/5 = Os, 6 = Esa, 7 = Esb; cols = half*1024 + g*512 + j

_CACHE = {}


def _build():
    """Build + compile the per-core Bass program (cached)."""
    if "nc" in _CACHE:
        return _CACHE["nc"]

    from concourse import bacc, tile, mybir

    f32 = mybir.dt.float32
    bf16 = mybir.dt.bfloat16

    nc = bacc.Bacc("TRN2", target_bir_lowering=False, debug=False)

    uv_d = nc.dram_tensor("uv", [128, NCOL], bf16, kind="ExternalInput")
    eo_d = nc.dram_tensor("eo", [128, 8 * 2048], bf16, kind="ExternalOutput")

    with tile.TileContext(nc) as tc, ExitStack() as ctx:
        wpool = ctx.enter_context(tc.tile_pool(name="w", bufs=1))
        opool = ctx.enter_context(tc.tile_pool(name="o", bufs=8))
        ppool = ctx.enter_context(tc.tile_pool(name="p", bufs=4, space="PSUM"))

        f32r = mybir.dt.float32r

        # warm-up operand needs no DMA: memset lands right after the prologue
        wu_t = wpool.tile([128, 512], f32, tag="wu")
        nc.gpsimd.memset(wu_t[:], 1.0)

        uv_t = wpool.tile([128, NCOL], bf16, tag="uv")
        for c0, c1 in XFERS:
            nc.sync.dma_start(uv_t[:, c0:c1], uv_d[:, c0:c1])

        # p-state warm-up: dummy matmuls keep the PE continuously busy from
        # the prologue until real operands arrive. The warm-up PSUM tile is
        # reused as the first phase's first accumulator (same engine, program
        # order) so the 4 PSUM double-buffers map 1:1 onto phase tiles and
        # phase p+2 only ever waits on phase p's casts.
        ps_w = ppool.tile([128, 2, 512], f32, tag="ps")
        for i in range(WARMUP_MM):
            nc.tensor.matmul(ps_w[:, i % 2, 0:128], wu_t[:, 0:128].bitcast(f32r),
                             wu_t[:, 0:128].bitcast(f32r), start=True, stop=True)

        # paired output tiles: two adjacent slots share one SBUF buffer and
        # drain as a single [128, 4096] transfer -> 8KB descriptors. Under a
        # saturated drain, SDMA engine 15 runs ~17% slower per packet than
        # the others and its backlog becomes the makespan tail; halving the
        # packet count halves that penalty.
        pair_state = {}

        def finish_phase(slot, pss, split):
            """Casts + output DMA for one phase's two PSUM tiles."""
            pair, idx = divmod(slot, 2)
            if idx == 0:
                pair_state[pair] = opool.tile([128, 2, 2, 2, 512], bf16,
                                              tag="out", name=f"out_{pair}")
            out_t = pair_state[pair]
            c0 = slot * 2048
            for half in range(2):
                ps = pss[half]
                if half == 0:
                    nc.scalar.copy(out_t[:, idx, half], ps[:])
                else:
                    nc.vector.tensor_copy(out_t[:, idx, half], ps[:])
            if split:
                # tail: previous slot + this one's halves on both rings
                nc.sync.dma_start(eo_d[:, c0 - 2048:c0], out_t[:, 0])
                nc.sync.dma_start(eo_d[:, c0:c0 + 1024], out_t[:, 1, 0])
                nc.scalar.dma_start(eo_d[:, c0 + 1024:c0 + 2048],
                                    out_t[:, 1, 1])
            elif idx == 1:
                nc.sync.dma_start(eo_d[:, c0 - 2048:c0 + 2048], out_t[:])

        def e_phase(kern2, sub, split=False):
            """Ea/Eb (or Esa/Esb): one 128-contraction sub-kernel."""
            slot = kern2 * 4 + 2 + sub
            cb = (C_B2, C_B3, C_B6, C_B7)[kern2 * 2 + sub]
            lhsT = uv_t[:, C_WE + (kern2 * 2 + sub) * 128:
                        C_WE + (kern2 * 2 + sub + 1) * 128]
            pss = [ppool.tile([128, 2, 512], f32, tag="ps",
                              name=f"ps_e{slot}_{h}") for h in range(2)]
            for half in range(2):
                for g in range(2):
                    nc.tensor.matmul(
                        pss[half][:, g, :],
                        lhsT,
                        uv_t[:, cb + half * 1024 + g * 512:
                             cb + half * 1024 + (g + 1) * 512],
                        start=True,
                        stop=True,
                    )
            finish_phase(slot, pss, split)

        def o_phase(kern2, kc, first=False):
            """O (or Os): 256-contraction, accumulated over two t-halves."""
            slot = kern2 * 4 + kc
            cb = C_B0 if kern2 == 0 else C_B4
            if first:
                pss = [ps_w,
                       ppool.tile([128, 2, 512], f32, tag="ps", name="ps_o0_1")]
            else:
                pss = [ppool.tile([128, 2, 512], f32, tag="ps",
                                  name=f"ps_o{slot}_{h}") for h in range(2)]
            for t in range(2):
                lhsT = uv_t[:, C_WO + t * 512 + kern2 * 256 + kc * 128:
                            C_WO + t * 512 + kern2 * 256 + (kc + 1) * 128]
                for half in range(2):
                    for g in range(2):
                        nc.tensor.matmul(
                            pss[half][:, g, :],
                            lhsT,
                            uv_t[:, cb + half * 2048 + t * 1024 + g * 512:
                                 cb + half * 2048 + t * 1024 + (g + 1) * 512],
                            start=(t == 0),
                            stop=(t == 1),
                        )
            finish_phase(slot, pss, split=False)

        o_phase(0, 0, first=True)  # O k0  (slot 0) -- heavy phases first
        o_phase(0, 1)            # O k1    (slot 1)
        e_phase(0, 0)            # Ea      (slot 2)
        e_phase(0, 1)            # Eb      (slot 3)
        o_phase(1, 0)            # Os k0   (slot 4)
        o_phase(1, 1)            # Os k1   (slot 5)
        e_phase(1, 0)            # Esa     (slot 6)
        e_phase(1, 1, split=True)  # Esb   (slot 7) -- tail on both rings

    nc.compile()
    _CACHE["nc"] = nc
    return nc


def kernel(x, wsin, wcos):
    from concourse.bass_utils import run_bass_kernel_spmd

    x = np.asarray(x, dtype=np.float32)
    wsin = np.asarray(wsin, dtype=np.float32)
    wcos = np.asarray(wcos, dtype=np.float32)

    nc = _build()

    # DFT kernels sliced from the provided (symmetric) matrices
    wO = wcos[1:512:2, 0:KD]            # cos, s = 2t+1      [256, 256]
    wOs = wsin[1:512:2, 0:KD]           # sin, s = 2t+1
    cEa = wcos[2:512:4, 0:128]          # cos, s = 4r+2      [128, 128]
    cEb = wcos[4:513:4, 0:128]          # cos, s = 4r+4
    sEa = wsin[2:512:4, 0:128]
    sEb = wsin[4:513:4, 0:128]
    wE_np = np.concatenate([cEa, cEb, sEa, sEb], axis=1).astype(BF16)
    oo = np.concatenate([wO, wOs], axis=1).astype(BF16)         # [256, 512]
    wO_np = np.ascontiguousarray(
        oo.reshape(2, 128, 512).transpose(1, 0, 2).reshape(128, 1024))

    # host fold + parity split (f32), then bf16
    xa = x[:, :, 1:512]
    xb = x[:, :, 1023:512:-1]
    u = xa + xb                         # u[s], s = 1..511
    v = xa - xb
    uvp = np.empty((B, F_FULL, 4, M), dtype=np.float32)
    uvp[:, :, 0, :255] = u[:, :, 1::2]  # ue: s = 2,4,..,510
    uvp[:, :, 0, 255] = x[:, :, 512]    # ue[255] <- u[512] = x[512]
    uvp[:, :, 1, :] = u[:, :, 0::2]     # uo: s = 1,3,..,511
    uvp[:, :, 2, :255] = v[:, :, 1::2]  # ve
    uvp[:, :, 2, 255] = 0.0
    uvp[:, :, 3, :] = v[:, :, 0::2]     # vo
    uvp_bf = uvp.astype(BF16)

    bpc = B // N_CORES
    in_maps = []
    for c in range(N_CORES):
        blk = uvp_bf[c * bpc:(c + 1) * bpc].reshape(F, 4, M)
        uv_c = np.empty((8, 128, F), dtype=BF16)
        for kern2, (iodd, ieven) in enumerate(((1, 0), (3, 2))):
            bb = kern2 * 4
            # odd-s data: [row, t] -> blocks [h][p, (tc, j)]
            ot = np.ascontiguousarray(blk[:, iodd, :].T)      # [256, F]
            uv_c[bb:bb + 2] = ot.reshape(2, 128, 2, 1024).transpose(
                2, 1, 0, 3).reshape(2, 128, 2048)
            # even-s data split by parity of t: flat [r, rows]
            ev = blk[:, ieven, :]                             # [F, 256]
            uv_c[bb + 2] = ev[:, 0::2].T                      # a: t = 2r
            uv_c[bb + 3] = ev[:, 1::2].T                      # b: t = 2r+1
        packed = np.concatenate(
            [wE_np, wO_np, uv_c[0], uv_c[1], uv_c[2], uv_c[3],
             uv_c[4], uv_c[5], uv_c[6], uv_c[7]], axis=1)
        in_maps.append({"uv": np.ascontiguousarray(packed)})

    res = run_bass_kernel_spmd(
        nc, in_maps, core_ids=list(range(N_CORES)), **_CACHE.get("run_kwargs", {})
    )
    kernel.last_results = res

    # host assembly: level-2 then level-1 butterflies, x[0] correction,
    # col 256, Hermitian mirror
    alt = np.where(np.arange(M) % 2 == 0, np.float32(1.0), np.float32(-1.0))
    altB = np.where(np.arange(128) % 2 == 0, np.float32(1.0),
                    np.float32(-1.0))
    out = np.empty((B, F_FULL, S), dtype=np.complex64)
    fv = out.view(np.float32).reshape(B, F_FULL, 2 * S)
    for c in range(N_CORES):
        b0 = c * bpc
        eo = np.asarray(res.results[c]["eo"]).reshape(
            128, 8, F).transpose(1, 0, 2)                     # [slot, k, row]
        O = np.concatenate([eo[0], eo[1]]).T.astype(np.float32)   # [F, 256]
        Ea = eo[2].T.astype(np.float32)                           # [F, 128]
        Eb = eo[3].T.astype(np.float32)
        Os = np.concatenate([eo[4], eo[5]]).T.astype(np.float32)
        Esa = eo[6].T.astype(np.float32)
        Esb = eo[7].T.astype(np.float32)
        blk32 = uvp[b0:b0 + bpc].reshape(F, 4, M)
        # level-2 butterflies: rebuild E, Es (k = 0..255)
        E = np.empty((F, 256), dtype=np.float32)
        E[:, 0:128] = Ea + Eb
        E[:, 129:256] = (Eb - Ea)[:, 127:0:-1]
        E[:, 128] = -(blk32[:, 0, 1::2] @ altB)
        Es = np.empty((F, 256), dtype=np.float32)
        Es[:, 0:128] = Esa + Esb
        Es[:, 129:256] = (Esa - Esb)[:, 127:0:-1]
        Es[:, 128] = blk32[:, 2, 0::2] @ altB
        # level-1 butterflies
        x0 = x[b0:b0 + bpc, :, 0].reshape(F, 1)
        reA = E + O
        reA += x0
        reB = E - O
        reB += x0
        imA = Es + Os
        np.negative(imA, out=imA)           # out.imag = -imag_raw
        imB = Es - Os
        fvb = fv[b0:b0 + bpc].reshape(F, 2 * S)
        fvb[:, 0:2 * KD:2] = reA            # real, k = 0..255
        fvb[:, 1:2 * KD:2] = imA
        fvb[:, 514:1026:2] = reB[:, ::-1]   # real, k = 257..512
        fvb[:, 515:1027:2] = imB[:, ::-1]
        # col 256: even-s cos run is (-1)^(t+1), odd-s sin run is (-1)^t
        fvb[:, 512] = x0[:, 0] - blk32[:, 0, :] @ alt
        fvb[:, 513] = -(blk32[:, 3, :] @ alt)
        # Hermitian mirror: out[k] = conj(out[1024-k]) for k = 513..1023
        fvb[:, 1026::2] = fvb[:, 1022:0:-2]
        fvb[:, 1027::2] = -fvb[:, 1023:1:-2]
    return out
